# revision 4
# baseline (speedup 1.0000x reference)
"""Trainium2 Bass kernel v2 for nn_Decoder_85916525789418 (GRU decoder with
per-scene self-attention), data-parallel over scenes across 8 NeuronCores.

v2 strategy (vs baseline): the baseline was bound by PE sequencer dispatch
(13.9k PE instructions) and balanced ~60k ns/step across all four engines.
This version:
  - fuses gate matmul pairs with fp8e4m3 DoubleRow matmuls (one instruction
    computes wia@a + whh@h etc.), cutting PE instructions and busy time,
  - uses Sigmoid ACT directly (r, 1-z) instead of tanh tricks,
  - applies the attention mask additively pre-exp via one PE inject (frees
    Pool), computes denominators with ONE full-width ones-matmul,
  - normalizes ctx after the ctx-matmul, fused into its PSUM->SBUF copy,
  - folds all biases into existing ops (tensor_scalar ptr / copy adds),
    skipping mu/std bias matmuls when the biases are zero.

Numerics: gate matmul operands (h, a, gx_n) are fp8e4m3; gx_r/gx_z injects
stay fp16; attention/fc/mu-std stay fp16/bf16; elementwise state fp16.
"""

import sys

for _p in ("/opt/trn_rl_repo",):
    if _p not in sys.path:
        sys.path.insert(0, _p)

import numpy as np
import ml_dtypes

import concourse.bass as bass
import concourse.mybir as mybir
from concourse import bacc, tile
from concourse.bass import ts, ds

NCORES = 8
B, PED, H, MLP, ZD, TT, NS, NP = 65536, 16, 128, 256, 32, 12, 6, 2
ZX = MLP + ZD          # 288
BL = B // NCORES       # 8192 peds per core
NB = 512               # peds per processing tile
NGRP = NB // 128       # 4 groups of 128 peds per tile

F32 = mybir.dt.float32
F32R = mybir.dt.float32r
BF16 = mybir.dt.bfloat16
FP16 = mybir.dt.float16
F8E4 = mybir.dt.float8e4
AF = mybir.ActivationFunctionType
OP = mybir.AluOpType
PM = mybir.MatmulPerfMode
F8NP = ml_dtypes.float8_e4m3fn
BF16NP = ml_dtypes.bfloat16

# X fp8 slot indices: [a8A, h8, a8B, gx2]
SL_A0, SL_H, SL_A1, SL_GX2 = 0, 1, 2, 3
NSLOT = 4


def build_module(bl=BL, t_steps=TT, use_bms=False, dbg=False, pipe_lag=1,
                 npre_mode='pe', tail_mode='zz'):
    nt = bl // NB
    nc = bacc.Bacc("TRN2", target_bir_lowering=False)

    # ---- DRAM I/O ----
    d_zxT = nc.dram_tensor("zxT", [ZX, bl], F32R, kind="ExternalInput")
    d_lsT = nc.dram_tensor("lsT", [NS, bl], F32R, kind="ExternalInput")
    d_fut8 = nc.dram_tensor("fut8", [t_steps * NP, bl], F8E4, kind="ExternalInput")
    # DoubleRow stationaries [128, 2, 128]: [kt0 | kt1]
    d_wr8e = nc.dram_tensor("wr8e", [128, 2, 128], F8E4, kind="ExternalInput")
    d_wr8o = nc.dram_tensor("wr8o", [128, 2, 128], F8E4, kind="ExternalInput")
    d_wz8e = nc.dram_tensor("wz8e", [128, 2, 128], F8E4, kind="ExternalInput")
    d_wz8o = nc.dram_tensor("wz8o", [128, 2, 128], F8E4, kind="ExternalInput")
    d_wn8 = nc.dram_tensor("wn8", [128, 128], F8E4, kind="ExternalInput")
    d_wn28 = nc.dram_tensor("wn28", [128, 2, 128], F8E4, kind="ExternalInput")
    d_wihxT = nc.dram_tensor("wihxT", [ZX, 3 * H], F32R, kind="ExternalInput")
    d_wdecT = nc.dram_tensor("wdecT", [ZX, H], F32R, kind="ExternalInput")
    d_wvelT = nc.dram_tensor("wvelT", [NS, NP], F32R, kind="ExternalInput")
    d_wattnT = nc.dram_tensor("wattnT", [2 * H, H], FP16, kind="ExternalInput")
    d_wms = nc.dram_tensor("wms", [H, 4], FP16, kind="ExternalInput")
    d_bmsb = nc.dram_tensor("bmsb", [128, 16], FP16, kind="ExternalInput")
    d_biasgx = nc.dram_tensor("biasgx", [H, 3], F32, kind="ExternalInput")
    d_bhhn = nc.dram_tensor("bhhn", [H, 1], F32, kind="ExternalInput")
    d_battn = nc.dram_tensor("battn", [H, 1], F32, kind="ExternalInput")
    d_bdec = nc.dram_tensor("bdec", [H, 1], F32, kind="ExternalInput")
    d_bvel = nc.dram_tensor("bvel", [NP, 1], F32, kind="ExternalInput")
    d_ident = nc.dram_tensor("ident", [128, 128], FP16, kind="ExternalInput")
    d_mask = nc.dram_tensor("maskadd", [128, NB], FP16, kind="ExternalInput")
    d_ones = nc.dram_tensor("ones128", [128, 128], BF16, kind="ExternalInput")
    # out[t, g, c, p]: c in (mu0, mu1, std0, std1)
    d_out = nc.dram_tensor("outT", [t_steps, bl // 128, 4, 128], F32,
                           kind="ExternalOutput")
    if dbg:
        d_dbg = {k: nc.dram_tensor(f"dbg_{k}", [128, NB],
                                   BF16 if k == "eT" else
                                   (F32 if k.startswith("ps") else FP16),
                                   kind="ExternalOutput")
                 for k in ("h0", "gx0", "rr", "zz", "ng", "hm", "eT", "ctxT",
                           "hf", "psr", "psz", "psn", "psn2", "h8q", "a8q")}

    with tile.TileContext(nc) as tc:
        with (
            tc.tile_pool(name="singles", bufs=1) as singles,
            tc.tile_pool(name="zxp", bufs=2) as zxp,
            tc.tile_pool(name="gwork", bufs=5) as gwork,
            tc.tile_pool(name="awork", bufs=4) as awork,
            tc.tile_pool(name="psum", bufs=1, space="PSUM") as psum,
        ):
            # ---- persistent SBUF state ----
            X = singles.tile([128, NSLOT, bl], F8E4)   # fp8 slots
            hT = singles.tile([128, bl], FP16)         # hidden state (post-fc)
            hmT = singles.tile([128, bl], FP16)        # GRU output (pre-fc)
            gx0f = singles.tile([128, bl], FP16)       # gx r-chunk (+bias)
            gx1f = singles.tile([128, bl], FP16)       # gx z-chunk (+bias)

            wr8e = singles.tile([128, 2, 128], F8E4)
            wr8o = singles.tile([128, 2, 128], F8E4)
            wz8e = singles.tile([128, 2, 128], F8E4)
            wz8o = singles.tile([128, 2, 128], F8E4)
            wn8 = singles.tile([128, 128], F8E4)
            wn28 = singles.tile([128, 2, 128], F8E4)
            wihx0 = singles.tile([128, 3 * H], F32R)
            wihx1 = singles.tile([128, 3 * H], F32R)
            wihx2 = singles.tile([ZX - 256, 3 * H], F32R)
            wdec0 = singles.tile([128, H], F32R)
            wdec1 = singles.tile([128, H], F32R)
            wdec2 = singles.tile([ZX - 256, H], F32R)
            wvelT = singles.tile([NS, NP], F32R)
            wat1 = singles.tile([H, H], FP16)
            wat2 = singles.tile([H, H], FP16)
            wms = singles.tile([H, 4], FP16)
            bmsb = singles.tile([128, 16], FP16)
            biasgx = singles.tile([H, 3], F32)
            bhhn = singles.tile([H, 1], F32)
            battn = singles.tile([H, 1], F32)
            bdec = singles.tile([H, 1], F32)
            bvel = singles.tile([NP, 1], F32)
            ident = singles.tile([128, 128], FP16)
            maskadd = singles.tile([128, NB], FP16)
            ones128 = singles.tile([128, 128], BF16)
            msbufs = [singles.tile([128, 16 * nt], F32, name=f"msb{j}")
                      for j in range(2)]

            for dst, src in [
                (wr8e, d_wr8e), (wr8o, d_wr8o), (wz8e, d_wz8e), (wz8o, d_wz8o),
                (wn8, d_wn8), (wn28, d_wn28),
                (wvelT, d_wvelT), (wms, d_wms),
                (bmsb, d_bmsb), (biasgx, d_biasgx), (bhhn, d_bhhn),
                (battn, d_battn), (bdec, d_bdec), (bvel, d_bvel),
                (ident, d_ident), (maskadd, d_mask), (ones128, d_ones),
            ]:
                nc.sync.dma_start(dst[:], src[:])
            nc.sync.dma_start(wat1[:], d_wattnT[0:128, :])
            nc.sync.dma_start(wat2[:], d_wattnT[128:256, :])
            nc.sync.dma_start(wihx0[:], d_wihxT[0:128, :])
            nc.sync.dma_start(wihx1[:], d_wihxT[128:256, :])
            nc.sync.dma_start(wihx2[:], d_wihxT[256:ZX, :])
            nc.sync.dma_start(wdec0[:], d_wdecT[0:128, :])
            nc.sync.dma_start(wdec1[:], d_wdecT[128:256, :])
            nc.sync.dma_start(wdec2[:], d_wdecT[256:ZX, :])

            # zero the a-slots fully; rows 0:NP are overwritten each step
            nc.vector.memset(X[:, SL_A0, :], 0.0)
            nc.vector.memset(X[:, SL_A1, :], 0.0)

            # ---- pre-loop: gx, h0, a0 ----
            for i in range(nt):
                sl = ts(i, NB)
                z0 = zxp.tile([128, NB], F32R, tag="z0")
                z1 = zxp.tile([128, NB], F32R, tag="z1")
                z2 = zxp.tile([ZX - 256, NB], F32R, tag="z2")
                nc.sync.dma_start(z0[:], d_zxT[0:128, sl])
                nc.sync.dma_start(z1[:], d_zxT[128:256, sl])
                nc.sync.dma_start(z2[:], d_zxT[256:ZX, sl])
                for oc in range(3):
                    ps = psum.tile([128, NB], F32, tag=["psc", "pden", "pctx"][oc])
                    nc.tensor.matmul(ps[:], wihx0[:, ts(oc, 128)],
                                     z0[:], start=True, stop=False)
                    nc.tensor.matmul(ps[:], wihx1[:, ts(oc, 128)],
                                     z1[:], start=False, stop=False)
                    nc.tensor.matmul(ps[:], wihx2[:, ts(oc, 128)],
                                     z2[:], start=False, stop=True)
                    dstv = (gx0f[:, sl], gx1f[:, sl],
                            X[:, SL_GX2, sl])[oc]
                    nc.scalar.activation(dstv, ps[:], AF.Identity,
                                         bias=biasgx[:, oc:oc + 1])
                # h0
                ps = psum.tile([128, NB], F32, tag="pfc")
                nc.tensor.matmul(ps[:], wdec0[:], z0[:],
                                 start=True, stop=False)
                nc.tensor.matmul(ps[:], wdec1[:], z1[:],
                                 start=False, stop=False)
                nc.tensor.matmul(ps[:], wdec2[:], z2[:],
                                 start=False, stop=True)
                nc.scalar.activation(hT[:, sl], ps[:], AF.Identity,
                                     bias=bdec[:, 0:1])
                nc.gpsimd.tensor_copy(X[:, SL_H, sl], hT[:, sl])
                # a0 -> fp8 a-slot A (t=0 is even)
                lst = zxp.tile([NS, NB], F32R, tag="ls")
                nc.sync.dma_start(lst[:], d_lsT[:, sl])
                psa = psum.tile([NP, NB], F32, tag="prn")
                nc.tensor.matmul(psa[:], wvelT[:], lst[:], start=True, stop=True)
                nc.vector.tensor_scalar(X[0:NP, SL_A0, sl], psa[:],
                                        bvel[:, 0:1], None, OP.add)

            # ---- time loop (software-pipelined: B lags A by PIPE_LAG tiles) ----
            PIPE_LAG = pipe_lag

            def emit_a(t, i):
                sl = ts(i, NB)
                if i == 0 and t > 0:
                    asl = SL_A0 if t % 2 == 0 else SL_A1
                    nc.sync.dma_start(X[0:NP, asl, :],
                                      d_fut8[ds(NP * (t - 1), NP), :])
                wr8 = wr8e if t % 2 == 0 else wr8o
                wz8 = wz8e if t % 2 == 0 else wz8o
                if True:
                    if t % 2 == 0:
                        mv_rz = X[:, SL_A0:SL_H + 1, sl]       # (a8A, h8)
                        mv_n2 = X[:, SL_A0:SL_GX2 + 1:3, sl]   # (a8A, gx2)
                    else:
                        mv_rz = X[:, SL_H:SL_A1 + 1, sl]       # (h8, a8B)
                        mv_n2 = X[:, SL_A1:SL_GX2 + 1, sl]     # (a8B, gx2)
                    psrz = psum.tile([128, 2 * NB], F32, tag="praz")
                    psn = psum.tile([128, NB], F32, tag="prn")
                    psn2 = psum.tile([128, NB], F32, tag="prn2")
                    nc.tensor.matmul(psrz[:, 0:NB], wr8[:], mv_rz,
                                     start=True, stop=False, perf_mode=PM.DoubleRow)
                    nc.tensor.matmul(psrz[:, 0:NB], ident[:], gx0f[:, sl],
                                     start=False, stop=True, skip_group_check=True)
                    nc.tensor.matmul(psrz[:, NB:2 * NB], wz8[:], mv_rz,
                                     start=True, stop=False, perf_mode=PM.DoubleRow)
                    nc.tensor.matmul(psrz[:, NB:2 * NB], ident[:], gx1f[:, sl],
                                     start=False, stop=True, skip_group_check=True)
                    nc.tensor.matmul(psn[:], wn8[:], X[:, SL_H, sl],
                                     start=True, stop=True)
                    nc.tensor.matmul(psn2[:], wn28[:], mv_n2,
                                     start=True, stop=False, perf_mode=PM.DoubleRow)

                    rrwz = gwork.tile([128, 2 * NB], FP16, tag="rrwz")
                    rr = rrwz[:, 0:NB]
                    wz = rrwz[:, NB:2 * NB]
                    tmp = gwork.tile([128, NB], FP16, tag="tmp")
                    ng = gwork.tile([128, NB], FP16, tag="ng")
                    dd = gwork.tile([128, NB], FP16, tag="dd")
                    m2 = gwork.tile([128, NB], FP16, tag="m2")

                    # r, 1-z via single-table tanh on the fused [128,1024] psum
                    nc.scalar.activation(rrwz[:], psrz[:], AF.Tanh, scale=0.5)
                    if tail_mode == 'zz':
                        zz = gwork.tile([128, NB], FP16, tag="zz")
                        nc.vector.tensor_scalar(zz[:], wz, -0.5, 0.5,
                                                OP.mult, OP.add)
                    if dbg and t == 0 and i == 0:
                        nc.sync.dma_start(d_dbg["h0"][:], hT[:, sl])
                        nc.sync.dma_start(d_dbg["gx0"][:], gx0f[:, sl])
                        nc.sync.dma_start(d_dbg["rr"][:], rr)
                        nc.sync.dma_start(d_dbg["zz"][:], zz[:])
                        for nm, pp in (("psr", psr), ("psz", psz),
                                       ("psn", psn), ("psn2", psn2)):
                            dt_ = gwork.tile([128, NB], F32, tag="dbgc",
                                             name=f"dbg{nm}")
                            nc.vector.tensor_copy(dt_[:], pp[:])
                            nc.sync.dma_start(d_dbg[nm][:], dt_[:])
                        for nm, slot in (("h8q", SL_H), ("a8q", SL_A0)):
                            dt_ = gwork.tile([128, NB], FP16, tag="dbgd",
                                             name=f"dbg{nm}")
                            nc.vector.tensor_copy(dt_[:], X[:, slot, sl])
                            nc.sync.dma_start(d_dbg[nm][:], dt_[:])
                    # tmp = (tanh+1) * (0.5 gh_n) = sigmoid * gh_n  [wn8 pre-halved]
                    nc.vector.scalar_tensor_tensor(tmp[:], rr, 1.0, psn[:],
                                                   OP.add, OP.mult)
                    if npre_mode == 'pe':
                        # accumulate tmp into psn2 on the PE (frees DVE)
                        nc.tensor.matmul(psn2[:], ident[:], tmp[:],
                                         start=False, stop=True,
                                         skip_group_check=True)
                    else:
                        npre = gwork.tile([128, NB], FP16, tag="npre")
                        nc.vector.tensor_tensor(npre[:], psn2[:], tmp[:], OP.add)
                        psn2 = npre
                    return dict(psn2=psn2, wz=wz,
                                zz=zz if tail_mode == 'zz' else None,
                                ng=ng, dd=dd, m2=m2, sl=sl)

            def emit_a2(t, i, st):
                sl, psn2, wz, zz = st["sl"], st["psn2"], st["wz"], st["zz"]
                ng, dd, m2 = st["ng"], st["dd"], st["m2"]
                if True:
                    nc.scalar.activation(ng[:], psn2[:], AF.Tanh)
                    # h_mid = h + (1-z)(n-h)
                    nc.gpsimd.tensor_tensor(dd[:], ng[:], hT[:, sl], OP.subtract)
                    if tail_mode == 'stt':
                        # m2 = (wz-1)(n-h); h_mid = -0.5*m2 + h  (stt illegal on Pool)
                        nc.vector.scalar_tensor_tensor(m2[:], wz, 1.0, dd[:],
                                                       OP.subtract, OP.mult)
                        nc.vector.scalar_tensor_tensor(hmT[:, sl], m2[:], -0.5,
                                                       hT[:, sl], OP.mult, OP.add)
                    else:
                        nc.gpsimd.tensor_tensor(m2[:], zz[:], dd[:], OP.mult)
                        nc.vector.tensor_tensor(hmT[:, sl], hT[:, sl], m2[:],
                                                OP.add)
                    if dbg and t == 0 and i == 0:
                        nc.sync.dma_start(d_dbg["ng"][:], ng[:])
                        nc.sync.dma_start(d_dbg["hm"][:], hmT[:, sl])

            def emit_b(t, i):
                sl = ts(i, NB)
                msbuf = msbufs[t % 2]
                if True:
                    # --- phase B: attention + fc + outputs ---
                    pssc = psum.tile([128, NB], F32, tag="psc")
                    eT = awork.tile([128, NB], BF16, tag="eT")
                    pden = psum.tile([128, NB], F32, tag="pden")
                    rdenB = awork.tile([128, NB], F32, tag="rden")
                    hrow = awork.tile([128, NB], BF16, tag="hrow")
                    ctxT = awork.tile([128, NB], FP16, tag="ctxT")

                    # mask first (full-bank start=True), then accumulate scores
                    nc.tensor.matmul(pssc[:], ident[:], maskadd[:],
                                     start=True, stop=False, skip_group_check=True)
                    for g in range(NGRP):
                        go = ts(g, 128)
                        ab = ds(i * NB + g * 128, 128)
                        nc.tensor.matmul(pssc[:, go], hmT[:, ab], hmT[:, ab],
                                         start=False, stop=(g == NGRP - 1),
                                         skip_group_check=True)
                    nc.scalar.activation(eT[:], pssc[:], AF.Exp)
                    nc.tensor.matmul(pden[:], ones128[:], eT[:],
                                     start=True, stop=True)
                    nc.vector.reciprocal_approx_fast(rdenB[:], pden[:])

                    pstr = psum.tile([128, NB], FP16, tag="pden")
                    for g in range(NGRP):
                        go = ts(g, 128)
                        ab = ds(i * NB + g * 128, 128)
                        nc.tensor.transpose(pstr[:, go], hmT[:, ab], ident[:])
                    nc.scalar.copy(hrow[:], pstr[:])

                    pctx = psum.tile([128, NB], F32, tag="pctx")
                    for g in range(NGRP):
                        go = ts(g, 128)
                        nc.tensor.matmul(pctx[:, go], hrow[:, go], eT[:, go],
                                         start=True, stop=True)
                    nc.vector.tensor_tensor(ctxT[:], pctx[:], rdenB[:],
                                            OP.mult)
                    if dbg and t == 0 and i == 0:
                        nc.sync.dma_start(d_dbg["eT"][:], eT[:])
                        nc.sync.dma_start(d_dbg["ctxT"][:], ctxT[:])

                    psfc = psum.tile([128, NB], F32, tag="pfc")
                    nc.tensor.matmul(psfc[:], wat1[:], hmT[:, sl],
                                     start=True, stop=False)
                    nc.tensor.matmul(psfc[:], wat2[:], ctxT[:],
                                     start=False, stop=True)
                    nc.vector.tensor_scalar(hT[:, sl], psfc[:], battn[:, 0:1],
                                            None, OP.add)
                    nc.gpsimd.tensor_copy(X[:, SL_H, sl], hT[:, sl])
                    if dbg and t == 0 and i == 0:
                        nc.sync.dma_start(d_dbg["hf"][:], hT[:, sl])

                    psms = psum.tile([128, NGRP * 4], F32, tag="pfc")
                    for g in range(NGRP):
                        ab = ds(i * NB + g * 128, 128)
                        nc.tensor.matmul(psms[:, ts(g, 4)], hT[:, ab], wms[:],
                                         start=True, stop=True)
                    nc.vector.tensor_tensor(msbuf[:, ds(16 * i, 16)], psms[:],
                                            bmsb[:], OP.add)
                if i == nt - 1:
                    # std cols -> exp(0.5*x); one DMA for the timestep
                    nc.scalar.activation(
                        msbuf[:].rearrange("p (i c) -> p i c", c=4)[:, :, 2:4],
                        msbuf[:].rearrange("p (i c) -> p i c", c=4)[:, :, 2:4],
                        AF.Exp, scale=0.5)
                    nc.sync.dma_start(
                        d_out[t].rearrange("g c p -> p (g c)"), msbuf[:])

            seq = [(t, i) for t in range(t_steps) for i in range(nt)]
            for k in range(len(seq) + PIPE_LAG):
                if k < len(seq):
                    st = emit_a(*seq[k])
                    emit_a2(*seq[k], st)
                if k >= PIPE_LAG:
                    emit_b(*seq[k - PIPE_LAG])

    nc.compile()
    return nc


def _host_pack(inputs, bl=BL, t_steps=TT, ncores=NCORES):
    """Slice + lay out the full inputs into per-core in_maps."""
    f32 = np.float32
    enc = np.asarray(inputs["enc_h_feat"], f32)
    zz = np.asarray(inputs["z"], f32)
    ls = np.asarray(inputs["last_state"], f32)
    fut = np.asarray(inputs["fut_state"], f32)
    W_dec = np.asarray(inputs["W_dec"], f32); b_dec = np.asarray(inputs["b_dec"], f32)
    W_vel = np.asarray(inputs["W_vel"], f32); b_vel = np.asarray(inputs["b_vel"], f32)
    W_ih = np.asarray(inputs["W_ih"], f32); b_ih = np.asarray(inputs["b_ih"], f32)
    W_hh = np.asarray(inputs["W_hh"], f32); b_hh = np.asarray(inputs["b_hh"], f32)
    W_attn = np.asarray(inputs["W_attn"], f32); b_attn = np.asarray(inputs["b_attn"], f32)
    W_mu = np.asarray(inputs["W_mu"], f32); b_mu = np.asarray(inputs["b_mu"], f32)
    W_std = np.asarray(inputs["W_std"], f32); b_std = np.asarray(inputs["b_std"], f32)

    zxT = np.ascontiguousarray(np.concatenate([enc, zz], axis=1).T)      # [288, B]
    lsT = np.ascontiguousarray(ls.T)                                     # [6, B]
    fut8 = np.ascontiguousarray(
        fut.transpose(0, 2, 1)).reshape(t_steps * NP, -1).astype(F8NP)

    W_ia = W_ih[:, ZX:]                                                  # [384, 2]
    whhT = np.ascontiguousarray(W_hh.T)                                  # [128, 384]

    def pad_wia(chunk):
        w = np.zeros((128, 128), f32)
        w[0:NP, :] = W_ia[chunk * 128:(chunk + 1) * 128, :].T
        return w

    def dr_pack(k0, k1):
        return np.ascontiguousarray(
            np.stack([k0, k1], axis=1)).astype(F8NP)     # [128, 2, 128]

    whh_r = whhT[:, 0:128]; whh_z = whhT[:, 128:256]; whh_n = whhT[:, 256:384]
    wr8e = dr_pack(pad_wia(0), whh_r)
    wr8o = dr_pack(whh_r, pad_wia(0))
    wz8e = dr_pack(pad_wia(1), whh_z)
    wz8o = dr_pack(whh_z, pad_wia(1))
    wn8 = np.ascontiguousarray(0.5 * whh_n).astype(F8NP)
    wn28 = dr_pack(pad_wia(2), np.eye(128, dtype=f32))

    wihxT = np.ascontiguousarray(W_ih[:, :ZX].T)                         # [288, 384]
    wdecT = np.ascontiguousarray(W_dec.T)                                # [288, 128]
    wvelT = np.ascontiguousarray(W_vel.T)                                # [6, 2]
    wattnT = np.ascontiguousarray(W_attn.T).astype(np.float16)           # [256, 128]
    wms = np.ascontiguousarray(
        np.concatenate([W_mu, W_std], axis=0).T).astype(np.float16)      # [128, 4]
    bms = np.concatenate([b_mu, b_std])                                  # [4]
    bmsb = np.tile(bms.reshape(1, 4), (128, 4)).astype(np.float16)       # [128, 16]
    biasgx = np.stack([
        b_ih[0:128] + b_hh[0:128],
        b_ih[128:256] + b_hh[128:256],
        b_ih[256:384],
    ], axis=1).astype(f32)                                               # [128, 3]
    bhhn = b_hh[256:384].reshape(H, 1).astype(f32)
    battn2 = b_attn.reshape(H, 1).astype(f32)
    bdec2 = b_dec.reshape(H, 1).astype(f32)
    bvel2 = b_vel.reshape(NP, 1).astype(f32)
    ident = np.eye(128, dtype=np.float16)
    blk1 = np.kron(np.eye(128 // PED, dtype=f32), np.ones((PED, PED), f32))
    maskadd = np.tile(-70.0 - 50.0 * (1.0 - blk1), (1, NB // 128)).astype(np.float16)
    ones128 = np.ones((128, 128), f32).astype(BF16NP)

    assert np.all(b_hh[256:384] == 0.0), "nonzero b_hh n-chunk unsupported in v2"

    shared = dict(wr8e=wr8e, wr8o=wr8o, wz8e=wz8e, wz8o=wz8o, wn8=wn8,
                  wn28=wn28, wihxT=wihxT, wdecT=wdecT, wvelT=wvelT,
                  wattnT=wattnT, wms=wms, bmsb=bmsb, biasgx=biasgx,
                  bhhn=bhhn, battn=battn2, bdec=bdec2, bvel=bvel2,
                  ident=ident, maskadd=maskadd, ones128=ones128)
    in_maps = []
    for c in range(ncores):
        sl = slice(c * bl, (c + 1) * bl)
        m = dict(shared)
        m["zxT"] = np.ascontiguousarray(zxT[:, sl])
        m["lsT"] = np.ascontiguousarray(lsT[:, sl])
        m["fut8"] = np.ascontiguousarray(fut8[:, sl])
        in_maps.append(m)
    return in_maps


def _assemble(results, bl=BL, t_steps=TT):
    outs = np.concatenate([r["outT"] for r in results], axis=1)  # [T, B/128, 4, 128]
    o = outs.transpose(0, 1, 3, 2).reshape(t_steps, -1, 4)       # [T, B, 4]
    mus = np.ascontiguousarray(o[:, :, 0:2])
    stds = np.ascontiguousarray(o[:, :, 2:4])
    return mus, stds


_NC_CACHE = {}


def run_kernel(inputs, trace=False, **kw):
    from concourse.bass_utils import run_bass_kernel_spmd
    key = "full"
    if key not in _NC_CACHE:
        _NC_CACHE[key] = build_module()
    nc = _NC_CACHE[key]
    in_maps = _host_pack(inputs)
    res = run_bass_kernel_spmd(nc, in_maps, core_ids=list(range(NCORES)),
                               trace=trace, **kw)
    mus, stds = _assemble(res.results)
    return mus, stds, res


def kernel(**inputs):
    mus, stds, _ = run_kernel(inputs)
    return mus, stds


if __name__ == "__main__":
    pass
